# revision 1
# baseline (speedup 1.0000x reference)
"""Trainium2 Bass kernel for nn_AttentionWeight (GAT edge softmax).

out[e,h] = softmax_over_dst_segments(relu(el[src]+er[dst]+ee[etype]))

Math used on device:
  exp(relu(x)) = max(exp(x), 1)  and  exp(x) = exp(el+ee)*exp(er)
  y := exp(relu(x)) - 1 = max(exp(el+ee)*exp(er) - 1, 0)
  segment_sum(exp(relu(x))) = sum(y) + deg   (padding slots give y = 0)
  out = (y + 1) * reciprocal(segment_sum)    (softmax is shift-invariant, the
                                              reference's max-subtraction is
                                              only for numerical range; values
                                              here are O(1) so it is not needed)

Distribution (8 NeuronCores):
  Launch A: node-sharded projections. Core s owns nodes [12500s, 12500(s+1)):
    el/er = feat @ (W_fc contracted with attn_l/attn_r) -> exp'd; the tiny
    edge-type table ee' = exp(contract(edge_emb@W_e, attn_e)); and the
    combined gather table el8[(n,t)] = el'[n]*ee'[t] for its node shard.
  Host: concatenates per-core el8 shards (pure relabeling, no arithmetic).
  Launch B: edge/dst-sharded softmax. Core c owns dst in [12500c, 12500(c+1)).
    Edges are dst-sorted and padded into [128 nodes x D_g] groups (nodes
    degree-sorted so groups are tight, ~1.5%% padding). One [128,1]-indexed
    indirect DMA gathers one slot column (128 rows of 32B) from el8; walrus
    miscompiles multi-index offset APs, so one instruction per column is the
    only correct form, and its ~1us SWDGE fixed cost on the Pool engine is
    the kernel's dominant term. Per group: multiply by broadcast er', the
    max(m-1,0) trick, a strided X-reduce for segment sums, reciprocal, and
    (y+1)*r, then store the padded slots.
  Host: scatters padded slots back to original edge order (indexing only).

All floating-point arithmetic happens on device; the host only shards,
permutes, concatenates and builds integer index/count arrays.
"""

import sys

sys.path.insert(0, "/opt/trn_rl_repo")

import numpy as np

import concourse.bass as bass
import concourse.bacc as bacc
import concourse.mybir as mybir
import concourse.tile as tile
from concourse.bass_utils import run_bass_kernel_spmd

# problem constants (hardcoded per harness contract)
N = 100000
E = 3200000
IN = 256
H = 8
O = 64
F = 64
T = 8
NCORES = 8
P = 128

NS = N // NCORES            # 12500 nodes per shard
NSP = 12544                 # padded to 128*98
G = NSP // P                # 98 groups of 128 nodes
ELFULL_ROWS = 128 * 785     # 100480: 8*12544=100352 real rows + pad
SENTINEL = 100352           # zero row in el_full -> el8 row SENTINEL*8 is 0
EL8_ROWS = ELFULL_ROWS * 8

FP = mybir.dt.float32
I32 = mybir.dt.int32

_timings = {}


# ---------------------------------------------------------------------------
# Launch A: projections
# ---------------------------------------------------------------------------

def _build_launch_a():
    nc = bacc.Bacc("TRN2", target_bir_lowering=False, debug=False,
                   num_devices=NCORES)
    featT = nc.dram_tensor("featT", [IN, NSP], FP, kind="ExternalInput")
    w_fc = nc.dram_tensor("w_fc", [IN, H * O], FP, kind="ExternalInput")
    attn_lr = nc.dram_tensor("attn_lr", [P, 2 * H * O], FP, kind="ExternalInput")
    edge_embT = nc.dram_tensor("edge_embT", [F, T], FP, kind="ExternalInput")
    w_e = nc.dram_tensor("w_e", [F, H * F], FP, kind="ExternalInput")
    attn_e = nc.dram_tensor("attn_e", [T, H * F], FP, kind="ExternalInput")
    erp = nc.dram_tensor("erp", [NSP, H], FP, kind="ExternalOutput")
    eep = nc.dram_tensor("eep", [T, H], FP, kind="ExternalOutput")
    el8s = nc.dram_tensor("el8s", [NSP * T, H], FP, kind="ExternalOutput")

    with tile.TileContext(nc) as tc:
        with (
            tc.tile_pool(name="sb", bufs=1) as sb,
            tc.tile_pool(name="mm", bufs=2) as mm,
            tc.tile_pool(name="ps", bufs=2, space="PSUM") as ps,
        ):
            # --- wl/wr: contract W_fc[i, h*O+o] with attn_l/r[h, o] -> [i, 2H]
            wfc_t = [sb.tile([P, H * O], FP, tag=f"wfc{c}", name=f"wfc{c}") for c in range(2)]
            for c in range(2):
                nc.sync.dma_start(wfc_t[c][:], w_fc[c * P:(c + 1) * P, :])
            alr_t = sb.tile([P, 2 * H * O], FP)
            nc.sync.dma_start(alr_t[:], attn_lr[:])
            wlr = [sb.tile([P, 2 * H], FP, tag=f"wlr{c}", name=f"wlr{c}") for c in range(2)]
            for c in range(2):
                for half in range(2):  # 0: attn_l, 1: attn_r
                    tmp = mm.tile([P, H * O], FP, tag="wtmp")
                    nc.vector.tensor_tensor(
                        tmp[:], wfc_t[c][:],
                        alr_t[:, half * H * O:(half + 1) * H * O],
                        mybir.AluOpType.mult)
                    nc.vector.tensor_reduce(
                        wlr[c][:, half * H:(half + 1) * H],
                        tmp[:].rearrange("p (h o) -> p h o", h=H),
                        mybir.AxisListType.X, mybir.AluOpType.add)

            # --- ee table: (edge_emb @ W_e) [T, H*F] contract attn_e -> [T, H]
            embT_t = sb.tile([F, T], FP)
            nc.sync.dma_start(embT_t[:], edge_embT[:])
            we_t = sb.tile([F, H * F], FP)
            nc.sync.dma_start(we_t[:], w_e[:])
            ae_t = sb.tile([T, H * F], FP)
            nc.sync.dma_start(ae_t[:], attn_e[:])
            proj_ps = ps.tile([T, H * F], FP)
            nc.tensor.matmul(proj_ps[:], lhsT=embT_t[:], rhs=we_t[:],
                             start=True, stop=True)
            proj_sb = sb.tile([T, H * F], FP)
            nc.vector.tensor_tensor(
                proj_sb[:], proj_ps[:], ae_t[:],
                mybir.AluOpType.mult)
            ee_sb = sb.tile([T, H], FP)
            nc.vector.tensor_reduce(
                ee_sb[:], proj_sb[:].rearrange("t (h f) -> t h f", h=H),
                mybir.AxisListType.X, mybir.AluOpType.add)
            eep_sb = sb.tile([T, H], FP)
            nc.scalar.activation(eep_sb[:], ee_sb[:],
                                 mybir.ActivationFunctionType.Exp)
            nc.sync.dma_start(eep[:], eep_sb[:])

            # --- el/er for the shard: node ln = p*G + tt handled by
            #     (tile tt, psum partition p)
            ftT = [sb.tile([P, NSP], FP, tag=f"ft{c}", name=f"ft{c}") for c in range(2)]
            for c in range(2):
                nc.sync.dma_start(ftT[c][:], featT[c * P:(c + 1) * P, :])
            elr = sb.tile([P, G, 2 * H], FP)
            # batch 32 node-tiles per single-bank PSUM tile ([128, 512] f32);
            # accumulation stays strictly sequential per 16-col slice (the
            # HW-verified pattern) -- only the exp drain is batched per bank.
            SLICES = 32
            tt = 0
            while tt < G:
                nsl = min(SLICES, G - tt)
                bank = ps.tile([P, SLICES * 2 * H], FP, tag="bank")
                for j in range(nsl):
                    sl = bank[:, j * 2 * H:(j + 1) * 2 * H]
                    for c in range(2):
                        lhsT = ftT[c][:].rearrange("i (p t) -> i t p", p=P)[:, tt + j, :]
                        nc.tensor.matmul(sl, lhsT=lhsT, rhs=wlr[c][:],
                                         start=(c == 0), stop=(c == 1))
                nc.scalar.activation(
                    elr[:, tt:tt + nsl, :],
                    bank[:, :nsl * 2 * H].rearrange("p (t h) -> p t h", h=2 * H),
                    mybir.ActivationFunctionType.Exp)
                tt += nsl
            # write out: partition p holds nodes [G*p, G*(p+1))
            nc.sync.dma_start(
                erp[:].rearrange("(p t) h -> p t h", p=P), elr[:, :, H:2 * H])
            # el8 shard: row (ln*T + t) = el'[ln] * ee'[t]
            eeb = sb.tile([P, T * H], FP)
            nc.sync.dma_start(
                eeb[:],
                eep[:].rearrange("t h -> (t h)").unsqueeze(0)
                .to_broadcast([P, T * H]))
            blk = sb.tile([P, G, T, H], FP)
            nc.vector.tensor_tensor(
                blk[:],
                elr[:, :, 0:H].unsqueeze(2).to_broadcast([P, G, T, H]),
                eeb[:].rearrange("p (t h) -> p t h", t=T).unsqueeze(1)
                .to_broadcast([P, G, T, H]),
                mybir.AluOpType.mult)
            nc.sync.dma_start(
                el8s[:].rearrange("(p g t) h -> p g t h", p=P, t=T), blk[:])

    nc.compile()
    return nc


# ---------------------------------------------------------------------------
# Launch B: edge softmax
# ---------------------------------------------------------------------------

def _build_launch_b(gds, ktot):
    """gds: per-group slot width D_g (len G); ktot = sum(gds)."""
    nc = bacc.Bacc("TRN2", target_bir_lowering=False, debug=False,
                   num_devices=NCORES)
    el8 = nc.dram_tensor("el8", [EL8_ROWS, H], FP, kind="ExternalInput")
    er_grid = nc.dram_tensor("er_grid", [P, G * H], FP, kind="ExternalInput")
    deg = nc.dram_tensor("deg", [P, G], FP, kind="ExternalInput")
    idx = nc.dram_tensor("idx", [P, ktot], I32, kind="ExternalInput")
    out = nc.dram_tensor("out", [P, ktot * H], FP, kind="ExternalOutput")

    with tile.TileContext(nc) as tc:
        # gather + softmax chain, one group of 128 dst nodes at a time
        with (
            tc.tile_pool(name="cst", bufs=1) as cst,
            tc.tile_pool(name="gp", bufs=3) as gp,
            tc.tile_pool(name="yp", bufs=3) as yp,
            tc.tile_pool(name="ip", bufs=3) as ip,
            tc.tile_pool(name="sp", bufs=3) as sp,
        ):
            er_sb = cst.tile([P, G, H], FP)
            nc.sync.dma_start(er_sb[:],
                              er_grid[:].rearrange("p (g h) -> p g h", g=G))
            deg_sb = cst.tile([P, G], FP)
            nc.sync.dma_start(deg_sb[:], deg[:])

            k0 = 0
            for g in range(len(gds)):
                dd = gds[g]
                idx_t = ip.tile([P, dd], I32, tag="idx")
                nc.sync.dma_start(idx_t[:], idx[:, k0:k0 + dd])
                g_t = gp.tile([P, dd, H], FP, tag="g")
                for k in range(dd):
                    nc.gpsimd.indirect_dma_start(
                        out=g_t[:, k, :],
                        out_offset=None,
                        in_=el8[:],
                        in_offset=bass.IndirectOffsetOnAxis(
                            ap=idx_t[:, k:k + 1], axis=0),
                    )
                # m = g * er ; y = max(m - 1, 0)
                y_t = yp.tile([P, dd, H], FP, tag="y")
                nc.vector.tensor_tensor(
                    y_t[:], g_t[:],
                    er_sb[:, g, :].unsqueeze(1).to_broadcast([P, dd, H]),
                    mybir.AluOpType.mult)
                nc.vector.tensor_scalar(y_t[:], y_t[:], 1.0, 0.0,
                                        mybir.AluOpType.subtract,
                                        mybir.AluOpType.max)
                # s = sum_d y + deg ; r = 1/s
                sums = sp.tile([P, H], FP, tag="sums")
                nc.vector.tensor_reduce(
                    sums[:], y_t[:].rearrange("p d h -> p h d"),
                    mybir.AxisListType.X, mybir.AluOpType.add)
                s_t = sp.tile([P, H], FP, tag="s")
                nc.vector.tensor_tensor(
                    s_t[:], sums[:],
                    deg_sb[:, g:g + 1].to_broadcast([P, H]),
                    mybir.AluOpType.add)
                r_t = sp.tile([P, H], FP, tag="r")
                nc.vector.reciprocal(r_t[:], s_t[:])
                # out = (y + 1) * r   (into the gather tile, then store)
                nc.vector.scalar_tensor_tensor(
                    g_t[:], y_t[:], 1.0,
                    r_t[:].unsqueeze(1).to_broadcast([P, dd, H]),
                    mybir.AluOpType.add, mybir.AluOpType.mult)
                nc.sync.dma_start(
                    out[:, k0 * H:(k0 + dd) * H],
                    g_t[:].rearrange("p k h -> p (k h)"))
                k0 += dd

    nc.compile()
    return nc


# ---------------------------------------------------------------------------
# Host orchestration
# ---------------------------------------------------------------------------

def kernel(feat, etype, src, dst, W_fc, edge_emb, W_e, attn_l, attn_r, attn_e):
    feat = np.asarray(feat)
    etype = np.asarray(etype).astype(np.int64)
    src = np.asarray(src).astype(np.int64)
    dst = np.asarray(dst).astype(np.int64)
    W_fc = np.asarray(W_fc)
    edge_emb = np.asarray(edge_emb)
    W_e = np.asarray(W_e)
    attn_l = np.asarray(attn_l)
    attn_r = np.asarray(attn_r)
    attn_e = np.asarray(attn_e)

    # ---------------- Launch A ----------------
    nc_a = _build_launch_a()
    attn_lr = np.concatenate(
        [attn_l.reshape(1, H * O), attn_r.reshape(1, H * O)], axis=1)
    in_maps_a = []
    for s in range(NCORES):
        featT_s = np.zeros((IN, NSP), np.float32)
        featT_s[:, :NS] = feat[s * NS:(s + 1) * NS].T
        in_maps_a.append({
            "featT": featT_s,
            "w_fc": W_fc.astype(np.float32),
            "attn_lr": np.broadcast_to(attn_lr.astype(np.float32), (P, 2 * H * O)).copy(),
            "edge_embT": np.ascontiguousarray(edge_emb.T.astype(np.float32)),
            "w_e": W_e.astype(np.float32),
            "attn_e": np.broadcast_to(attn_e.reshape(1, H * F).astype(np.float32), (T, H * F)).copy(),
        })
    res_a = run_bass_kernel_spmd(nc_a, in_maps_a, core_ids=list(range(NCORES)))

    # el8 row(n, t) = (12544*(n//12500) + n%12500)*T + t ; tail rows zero pad
    el8_full = np.zeros((EL8_ROWS, H), np.float32)
    er_all = np.zeros((NCORES, NSP, H), np.float32)
    for s in range(NCORES):
        el8_full[s * NSP * T:(s + 1) * NSP * T] = res_a.results[s]["el8s"]
        er_all[s] = res_a.results[s]["erp"]

    # ---------------- host index construction (integers only) -------------
    # edges to cores by dst range; dst-sort within core
    core_of = dst // NS
    order_all = np.argsort(core_of * (2 * N) + dst, kind="stable")

    per_core = []
    for c in range(NCORES):
        lo = np.searchsorted(core_of[order_all], c, side="left")
        hi = np.searchsorted(core_of[order_all], c, side="right")
        per_core.append(order_all[lo:hi])

    # degree-sorted node grouping per core (shared chunk structure)
    node_perm = np.zeros((NCORES, NSP), np.int64)   # grid pos -> local node
    degrees = np.zeros((NCORES, NSP), np.int64)
    for c in range(NCORES):
        e_ids = per_core[c]
        ld = dst[e_ids] - c * NS
        cnt = np.bincount(ld, minlength=NSP)
        perm = np.argsort(cnt, kind="stable")       # ascending degree
        node_perm[c] = perm
        degrees[c] = cnt[perm]

    # groups: grid position (p, g) -> node_perm[g*128 + p]  (sorted order runs
    # down the group-axis first so consecutive groups have similar degrees)
    # group g covers sorted positions [g*128, (g+1)*128)
    gmax = degrees.reshape(NCORES, G, P).max(axis=2).max(axis=0)  # [G]

    # per-group slot width
    gds = [int(max(d, 1)) for d in gmax]
    ktot = sum(gds)

    nc_b = _build_launch_b(gds, ktot)

    # per-core B inputs
    in_maps_b = []
    slot_edge = np.full((NCORES, P, ktot), -1, np.int64)  # slot -> edge id
    for c in range(NCORES):
        e_ids = per_core[c]                      # dst-sorted edge ids
        ld = dst[e_ids] - c * NS
        cnt = np.bincount(ld, minlength=NSP)
        starts = np.concatenate([[0], np.cumsum(cnt)])
        perm = node_perm[c]
        inv_sorted_pos = np.empty(NSP, np.int64)
        inv_sorted_pos[perm] = np.arange(NSP)

        colbase = np.concatenate([[0], np.cumsum(gds)[:-1]]).astype(np.int64)

        nodes_pg = perm.reshape(G, P)                    # grid (g, p) -> node
        er_grid = er_all[c][nodes_pg].transpose(1, 0, 2)  # [P, G, H]
        deg_np = np.maximum(cnt[nodes_pg], 1).T.astype(np.float32)  # [P, G]

        # vectorized per-edge slot assignment (e_ids is dst-sorted)
        ld = dst[e_ids] - c * NS
        rank = np.arange(len(e_ids)) - starts[ld]
        spos = inv_sorted_pos[ld]
        gg_ = spos // P
        pp_ = spos % P
        cols = colbase[gg_] + rank
        rows = (src[e_ids] // NS) * NSP + (src[e_ids] % NS)
        idx_np = np.full((P, ktot), SENTINEL * T, np.int64)
        idx_np[pp_, cols] = rows * T + etype[e_ids]
        slot_edge[c, pp_, cols] = e_ids

        in_maps_b.append({
            "el8": el8_full,
            "er_grid": er_grid.reshape(P, G * H),
            "deg": deg_np,
            "idx": idx_np.astype(np.int32),
        })

    res_b = run_bass_kernel_spmd(nc_b, in_maps_b, core_ids=list(range(NCORES)))

    # ---------------- unshard ----------------
    out = np.zeros((E, H), np.float32)
    for c in range(NCORES):
        o_c = res_b.results[c]["out"].reshape(P, ktot, H)
        mask = slot_edge[c] >= 0
        out[slot_edge[c][mask]] = o_c[mask]

    # timing estimate via the cost-model simulator (no NTFF profiling
    # available under this axon client; see test.py)
    try:
        from concourse.timeline_sim import TimelineSim
        _timings["A_ns"] = TimelineSim(nc_a).simulate()
        _timings["B_ns"] = TimelineSim(nc_b).simulate()
    except Exception as ex:  # timing must never break correctness
        _timings["error"] = repr(ex)

    return out

